# revision 8
# baseline (speedup 1.0000x reference)
"""Trainium2 Bass kernel for nn_AppearanceTrackletEmb (dense_transformer).

Pure data-parallel: batch B=256 sharded across 8 NeuronCores (32 pairs /
64 independent sequences per core). All weights replicated. Everything
(4 attention layers + pooling + classifier) runs on-chip in one NEFF.

Layout strategy (per core):
  - x is kept channel-major [C, tokens] on chip; the host pre-transposes
    the layer-0 input to [CIN_pad, tokens] (bf16) so no on-chip transposes
    are ever needed.
  - Attention uses the S' = k^T q orientation: S'[m,l] with softmax over
    the partition dim handled WITHOUT partition reductions by computing
    Z = ones^T @ E with a ones-matmul (broadcasts Z over partitions).
  - V is produced token-major [tokens, C] directly (x as lhsT), so the
    second attention matmul out2 = vT.T @ En needs no transposed operands.
  - Biases: layer-0 biases ride in an augmented ones-row of x/W; layer
    1-3 q/k biases fuse into the PSUM->SBUF activation (per-partition);
    layer 1-3 v biases are added with a rank-1 ones matmul.
"""

import sys

if "/opt/trn_rl_repo" not in sys.path:
    sys.path.insert(0, "/opt/trn_rl_repo")

from contextlib import ExitStack

import ml_dtypes
import numpy as np

import concourse.bass as bass
import concourse.tile as tile
from concourse import bacc, mybir
from concourse.bass_utils import run_bass_kernel_spmd

BF16 = mybir.dt.bfloat16
F32 = mybir.dt.float32
AF = mybir.ActivationFunctionType
ALU = mybir.AluOpType
AX = mybir.AxisListType

C = 512
L = 65
CIN = 2053
SCALE = 1.0 / float(np.sqrt(C))
N_CORES = 8


def build_graph(n_seq=64, G=8, KI=17):
    """Build the per-core Bass graph.

    n_seq: sequences per core (left+right), G: sequences per chunk,
    KI: number of 128-row k-tiles for the (padded) input channel dim.
    """
    CINP = KI * 128
    T = G * L          # tokens per chunk
    H = T // 2         # free-dim half (matmul N and PSUM bank limit)
    GH = G // 2        # sequences per half
    n_chunks = n_seq // G
    n_pairs = n_seq // 2
    CT = C // 128      # 4 channel tiles
    assert n_seq % G == 0 and G % 2 == 0
    assert H <= 512

    nc = bacc.Bacc(None)

    xT_d = nc.dram_tensor("xT", [CINP, n_seq * L], BF16, kind="ExternalInput")
    w0T_d = nc.dram_tensor("w0T", [3, CINP, C], BF16, kind="ExternalInput")
    wqT_d = nc.dram_tensor("wqT", [3, C, C], BF16, kind="ExternalInput")
    wkT_d = nc.dram_tensor("wkT", [3, C, C], BF16, kind="ExternalInput")
    wvT_d = nc.dram_tensor("wvT", [3, C, C], BF16, kind="ExternalInput")
    bqk_d = nc.dram_tensor("bqk", [128, CT, 6], F32, kind="ExternalInput")
    bvT_d = nc.dram_tensor("bvT", [3, C], BF16, kind="ExternalInput")
    w1T_d = nc.dram_tensor("w1T", [2 * C, 256], BF16, kind="ExternalInput")
    b1_d = nc.dram_tensor("b1", [128, 2], F32, kind="ExternalInput")
    w2T_d = nc.dram_tensor("w2T", [256, 2], BF16, kind="ExternalInput")
    b2_d = nc.dram_tensor("b2", [2, 1], F32, kind="ExternalInput")
    out_d = nc.dram_tensor("out", [2, n_pairs], F32, kind="ExternalOutput")

    with tile.TileContext(nc) as tc, ExitStack() as ctx:
        wpool = ctx.enter_context(tc.tile_pool(name="w", bufs=1))
        xpool = ctx.enter_context(tc.tile_pool(name="xt", bufs=2))
        apool = ctx.enter_context(tc.tile_pool(name="act", bufs=2))
        xlpool = ctx.enter_context(tc.tile_pool(name="xl", bufs=3))
        pj_ps = ctx.enter_context(tc.tile_pool(name="pjps", bufs=2, space="PSUM"))
        at_ps = ctx.enter_context(tc.tile_pool(name="atps", bufs=2, space="PSUM"))

        # --- persistent weights/constants ---
        w0T_sb = wpool.tile([128, 3, KI, C], BF16)
        nc.sync.dma_start(w0T_sb[:], w0T_d.rearrange("j (a p) c -> p j a c", p=128))
        wqT_sb = wpool.tile([128, 3, CT, C], BF16)
        nc.sync.dma_start(wqT_sb[:], wqT_d.rearrange("n (a p) c -> p n a c", p=128))
        wkT_sb = wpool.tile([128, 3, CT, C], BF16)
        nc.sync.dma_start(wkT_sb[:], wkT_d.rearrange("n (a p) c -> p n a c", p=128))
        wvT_sb = wpool.tile([128, 3, CT, C], BF16)
        nc.sync.dma_start(wvT_sb[:], wvT_d.rearrange("n (a p) c -> p n a c", p=128))
        bqk_sb = wpool.tile([128, CT, 6], F32)
        nc.sync.dma_start(bqk_sb[:], bqk_d[:])
        bvT_sb = wpool.tile([1, 3, C], BF16)
        nc.sync.dma_start(bvT_sb[:], bvT_d[None, :, :])
        w1T_sb = wpool.tile([128, 8, 256], BF16)
        nc.sync.dma_start(w1T_sb[:], w1T_d.rearrange("(a p) m -> p a m", p=128))
        b1_sb = wpool.tile([128, 2], F32)
        nc.sync.dma_start(b1_sb[:], b1_d[:])
        w2T_sb = wpool.tile([128, 2, 2], BF16)
        nc.sync.dma_start(w2T_sb[:], w2T_d.rearrange("(a p) m -> p a m", p=128))
        b2_sb = wpool.tile([2, 1], F32)
        nc.sync.dma_start(b2_sb[:], b2_d[:])

        ones65 = wpool.tile([65, 65], BF16)
        nc.vector.memset(ones65[:], 1.0)
        ones1 = wpool.tile([1, 65], BF16)
        nc.vector.memset(ones1[:], 1.0)

        P_sb = wpool.tile([128, CT, n_seq], BF16)
        h2_sb = wpool.tile([128, 2, n_pairs], BF16)
        y_sb = wpool.tile([2, n_pairs], F32)

        def proj_layer0(xt):
            qk = []
            for j in range(2):  # 0=q, 1=k (channel-major [C, T])
                dst = apool.tile([128, CT, T], BF16, tag=f"qk{j}")
                for ct in range(CT):
                    for h in range(2):
                        ps = pj_ps.tile([128, H], F32, tag="pj")
                        for ki in range(KI):
                            nc.tensor.matmul(
                                ps[:],
                                w0T_sb[:, j, ki, ct * 128:(ct + 1) * 128],
                                xt[:, ki, h * H:(h + 1) * H],
                                start=(ki == 0),
                                stop=(ki == KI - 1),
                            )
                        nc.scalar.activation(dst[:, ct, h * H:(h + 1) * H], ps[:], AF.Copy)
                qk.append(dst)
            vT = apool.tile([65, G, C], BF16, tag="vt")
            for s in range(G):  # token-major v per sequence
                ps = pj_ps.tile([65, C], F32, tag="pjv")
                for ki in range(KI):
                    nc.tensor.matmul(
                        ps[:],
                        xt[:, ki, s * L:(s + 1) * L],
                        w0T_sb[:, 2, ki, :],
                        start=(ki == 0),
                        stop=(ki == KI - 1),
                    )
                vr = apool.tile([65, C], F32, tag="vr")
                nc.scalar.activation(vr[:65, :], ps[:], AF.Relu, scale=0.99)
                nc.vector.scalar_tensor_tensor(vT[:65, s, :], ps[:], 0.01, vr[:65, :],
                                               op0=ALU.mult, op1=ALU.add)
            return qk[0], qk[1], vT

        def proj_layer(x, ly):
            qk = []
            for j, w in enumerate((wqT_sb, wkT_sb)):
                dst = apool.tile([128, CT, T], BF16, tag=f"qk{j}")
                for ct in range(CT):
                    for h in range(2):
                        ps = pj_ps.tile([128, H], F32, tag="pj")
                        for ki in range(CT):
                            nc.tensor.matmul(
                                ps[:],
                                w[:, ly, ki, ct * 128:(ct + 1) * 128],
                                x[:, ki, h * H:(h + 1) * H],
                                start=(ki == 0),
                                stop=(ki == CT - 1),
                            )
                        col = j * 3 + ly
                        nc.scalar.activation(
                            dst[:, ct, h * H:(h + 1) * H], ps[:],
                            AF.Identity, bias=bqk_sb[:, ct, col:col + 1], scale=1.0,
                        )
                qk.append(dst)
            vT = apool.tile([65, G, C], BF16, tag="vt")
            for s in range(G):
                ps = pj_ps.tile([65, C], F32, tag="pjv")
                for ki in range(CT):
                    nc.tensor.matmul(
                        ps[:],
                        x[:, ki, s * L:(s + 1) * L],
                        wvT_sb[:, ly, ki, :],
                        start=(ki == 0),
                        stop=False,
                    )
                nc.tensor.matmul(  # rank-1 bias: ones(65) x bv
                    ps[:], ones1[:], bvT_sb[:, ly, :], start=False, stop=True,
                )
                vr = apool.tile([65, C], F32, tag="vr")
                nc.scalar.activation(vr[:65, :], ps[:], AF.Relu, scale=0.99)
                nc.vector.scalar_tensor_tensor(vT[:65, s, :], ps[:], 0.01, vr[:65, :],
                                               op0=ALU.mult, op1=ALU.add)
            return qk[0], qk[1], vT

        def attention(q, k, vT, x_prev):
            E = apool.tile([65, T], BF16, tag="E")
            for s in range(G):
                ps = at_ps.tile([65, L], F32, tag="sp")
                for ct in range(CT):
                    nc.tensor.matmul(
                        ps[:],
                        k[:, ct, s * L:(s + 1) * L],
                        q[:, ct, s * L:(s + 1) * L],
                        start=(ct == 0),
                        stop=(ct == CT - 1),
                    )
                nc.scalar.activation(E[:65, s * L:(s + 1) * L], ps[:], AF.Exp, scale=SCALE)
            zr = apool.tile([65, T], F32, tag="zr")
            for h in range(2):
                ps = at_ps.tile([65, H], F32, tag="sp")
                nc.tensor.matmul(ps[:], ones65[:], E[:65, h * H:(h + 1) * H],
                                 start=True, stop=True)
                nc.vector.reciprocal(zr[:65, h * H:(h + 1) * H], ps[:])
            En = apool.tile([65, T], BF16, tag="En")
            nc.vector.tensor_tensor(En[:65, :], E[:65, :], zr[:65, :], op=ALU.mult)
            xn = xlpool.tile([128, CT, T], BF16, tag="x")
            for ct in range(CT):
                for h in range(2):
                    ps = at_ps.tile([128, H], F32, tag="o2")
                    for s4 in range(GH):
                        s = h * GH + s4
                        nc.tensor.matmul(
                            ps[:, s4 * L:(s4 + 1) * L],
                            vT[:65, s, ct * 128:(ct + 1) * 128],
                            En[:65, s * L:(s + 1) * L],
                            start=True,
                            stop=True,
                        )
                    if x_prev is None:
                        nc.vector.tensor_copy(xn[:, ct, h * H:(h + 1) * H], ps[:])
                    else:
                        nc.vector.tensor_add(
                            xn[:, ct, h * H:(h + 1) * H], ps[:],
                            x_prev[:, ct, h * H:(h + 1) * H],
                        )
            return xn

        def pool_chunk(x, c):
            for ct in range(CT):
                red = apool.tile([128, G], F32, tag="red")
                nc.vector.tensor_reduce(
                    red[:], x[:, ct, :].rearrange("p (s l) -> p s l", l=L),
                    axis=AX.X, op=ALU.add,
                )
                nc.scalar.activation(P_sb[:, ct, c * G:(c + 1) * G], red[:],
                                     AF.Copy, scale=1.0 / L)

        def classifier():
            for mt in range(2):
                ps = pj_ps.tile([128, n_pairs], F32, tag="pj")
                for ki in range(8):
                    rhs = (P_sb[:, ki, 0:n_pairs] if ki < CT
                           else P_sb[:, ki - CT, n_pairs:2 * n_pairs])
                    nc.tensor.matmul(ps[:], w1T_sb[:, ki, mt * 128:(mt + 1) * 128],
                                     rhs, start=(ki == 0), stop=(ki == 7))
                nc.scalar.activation(h2_sb[:, mt, :], ps[:], AF.Relu,
                                     bias=b1_sb[:, mt:mt + 1], scale=1.0)
            ps = at_ps.tile([2, n_pairs], F32, tag="sp")
            for ki in range(2):
                nc.tensor.matmul(ps[:], w2T_sb[:, ki, :], h2_sb[:, ki, :],
                                 start=(ki == 0), stop=(ki == 1))
            nc.scalar.activation(y_sb[:], ps[:], AF.Identity,
                                 bias=b2_sb[:], scale=1.0)
            nc.sync.dma_start(out_d[:], y_sb[:])

        for c in range(n_chunks):
            xt = xpool.tile([128, KI, T], BF16, tag="xt")
            nc.sync.dma_start(
                xt[:], xT_d[:, c * T:(c + 1) * T].rearrange("(a p) t -> p a t", p=128)
            )
            q, k, vT = proj_layer0(xt)
            x = attention(q, k, vT, None)
            for ly in range(3):
                q, k, vT = proj_layer(x, ly)
                x = attention(q, k, vT, x)
            pool_chunk(x, c)
        classifier()

    nc.finalize()
    return nc


def prep_weights(Wq0, bq0, Wk0, bk0, Wv0, bv0, Wq, bq, Wk, bk, Wv, bv,
                 W1, b1, W2, b2, KI=17):
    """Host-side weight prep shared by all cores."""
    bf = ml_dtypes.bfloat16
    CINP = KI * 128
    w0T = np.zeros((3, CINP, C), np.float32)
    for j, (W_, b_) in enumerate([(Wq0, bq0), (Wk0, bk0), (Wv0, bv0)]):
        w0T[j, :CIN, :] = np.asarray(W_, np.float32).T
        w0T[j, CIN, :] = np.asarray(b_, np.float32)
    wqT = np.ascontiguousarray(np.transpose(np.asarray(Wq, np.float32), (0, 2, 1)))
    wkT = np.ascontiguousarray(np.transpose(np.asarray(Wk, np.float32), (0, 2, 1)))
    wvT = np.ascontiguousarray(np.transpose(np.asarray(Wv, np.float32), (0, 2, 1)))
    # bqk[p, ct, col]: cols 0..2 = bq layers 1..3, 3..5 = bk layers 1..3
    bqk = np.zeros((128, C // 128, 6), np.float32)
    for ly in range(3):
        bqk[:, :, ly] = np.asarray(bq, np.float32)[ly].reshape(C // 128, 128).T
        bqk[:, :, 3 + ly] = np.asarray(bk, np.float32)[ly].reshape(C // 128, 128).T
    w1T = np.ascontiguousarray(np.asarray(W1, np.float32).T)
    b1h = np.ascontiguousarray(np.asarray(b1, np.float32).reshape(2, 128).T)
    w2T = np.ascontiguousarray(np.asarray(W2, np.float32).T)
    b2h = np.asarray(b2, np.float32).reshape(2, 1)
    return {
        "w0T": w0T.astype(bf),
        "wqT": wqT.astype(bf),
        "wkT": wkT.astype(bf),
        "wvT": wvT.astype(bf),
        "bqk": bqk,
        "bvT": np.asarray(bv, np.float32).astype(bf),
        "w1T": w1T.astype(bf),
        "b1": b1h,
        "w2T": w2T.astype(bf),
        "b2": b2h,
    }


def prep_xT(dl, dr, KI=17):
    """[n_pairs, L, CIN] left+right -> padded channel-major [CINP, n_seq*L] bf16.

    Sequence order: all left sequences then all right sequences.
    Row CIN is the ones-row that carries layer-0 biases.
    """
    bf = ml_dtypes.bfloat16
    CINP = KI * 128
    n_pairs = dl.shape[0]
    ntok = n_pairs * L
    xT = np.zeros((CINP, 2 * ntok), np.float32)
    xT[:CIN, :ntok] = np.asarray(dl, np.float32).reshape(ntok, CIN).T
    xT[:CIN, ntok:] = np.asarray(dr, np.float32).reshape(ntok, CIN).T
    xT[CIN, :] = 1.0
    return xT.astype(bf)


def _ensure_ntff_hook():
    """Provide antenv.axon_hooks with a ctypes NTFF profile hook if the
    image's antenv lacks it (bass_utils imports it unguarded when
    trace=True under axon)."""
    try:
        from antenv.axon_hooks import get_axon_ntff_profile_hook  # noqa: F401
        return
    except ImportError:
        pass
    import contextlib
    import ctypes
    import types

    import antenv

    mod = types.ModuleType("antenv.axon_hooks")
    holder = {"hook": None}
    mod.set_axon_ntff_profile_hook = lambda h: holder.update(hook=h)
    mod.get_axon_ntff_profile_hook = lambda: holder["hook"]
    sys.modules["antenv.axon_hooks"] = mod
    antenv.axon_hooks = mod

    so_path = "/opt/axon/libaxon_pjrt.so"
    try:
        lib = ctypes.CDLL(so_path)
    except OSError:
        return
    if not hasattr(lib, "axon_start_nrt_profile"):
        return
    lib.axon_start_nrt_profile.argtypes = [ctypes.POINTER(ctypes.c_int64),
                                           ctypes.c_size_t]
    lib.axon_start_nrt_profile.restype = ctypes.c_int64
    lib.axon_stop_nrt_profile.argtypes = [ctypes.c_char_p]
    lib.axon_stop_nrt_profile.restype = ctypes.c_int64

    @contextlib.contextmanager
    def _hook(output_dir, device_ids):
        import jax

        jax.devices()
        if device_ids:
            ids = (ctypes.c_int64 * len(device_ids))(*device_ids)
            rc = lib.axon_start_nrt_profile(ids, len(device_ids))
        else:
            rc = lib.axon_start_nrt_profile(None, 0)
        if rc != 0:
            raise RuntimeError(f"axon_start_nrt_profile rc={rc}")
        try:
            yield
        finally:
            n = lib.axon_stop_nrt_profile(str(output_dir).encode())
            print(f"ntff profile: {n} file(s) written to {output_dir}",
                  file=sys.stderr)

    holder["hook"] = _hook


_GRAPH_CACHE = {}


def _get_graph(n_seq, G, KI):
    key = (n_seq, G, KI)
    if key not in _GRAPH_CACHE:
        _GRAPH_CACHE[key] = build_graph(n_seq=n_seq, G=G, KI=KI)
    return _GRAPH_CACHE[key]


def kernel(dataleft, dataright, Wq0, bq0, Wk0, bk0, Wv0, bv0,
           Wq, bq, Wk, bk, Wv, bv, W1, b1, W2, b2):
    import os

    B = dataleft.shape[0]
    per = B // N_CORES
    nc = _get_graph(n_seq=2 * per, G=8, KI=17)
    wmap = prep_weights(Wq0, bq0, Wk0, bk0, Wv0, bv0, Wq, bq, Wk, bk, Wv, bv,
                        W1, b1, W2, b2)
    in_maps = []
    for i in range(N_CORES):
        m = dict(wmap)
        m["xT"] = prep_xT(dataleft[i * per:(i + 1) * per],
                          dataright[i * per:(i + 1) * per])
        in_maps.append(m)
    trace = bool(int(os.environ.get("KERNEL_TRACE", "0")))
    if trace:
        _ensure_ntff_hook()
    res = run_bass_kernel_spmd(nc, in_maps, core_ids=list(range(N_CORES)),
                               trace=trace)
    if trace and res.exec_time_ns is not None:
        print(f"HW exec time: {res.exec_time_ns} ns")
        kernel.last_exec_time_ns = res.exec_time_ns
        kernel.last_profile = res
    out = np.concatenate([np.ascontiguousarray(r["out"].T) for r in res.results], 0)
    return out.astype(np.float32)


# revision 11
# speedup vs baseline: 1.2141x; 1.2141x over previous
"""Trainium2 Bass kernel for nn_AppearanceTrackletEmb (dense_transformer).

Pure data-parallel: batch B=256 sharded across 8 NeuronCores (32 pairs /
64 independent sequences per core). All weights replicated. Everything
(4 attention layers + pooling + classifier) runs on-chip in one NEFF.

Layout strategy (per core):
  - x is kept channel-major [C, tokens] on chip; the host pre-transposes
    the layer-0 input to [CIN_pad, tokens] (bf16) so no on-chip transposes
    are ever needed.
  - Attention uses the S' = k^T q orientation: S'[m,l]. The softmax
    denominator Z = ones^T @ E is computed with a ones-matmul whose
    stationary operand is [65, 128], which broadcasts Z across all 128
    partitions; normalization is deferred to the out2 epilogue (a DVE
    multiply by 1/Z) so the PE never waits on the softmax chain.
  - V is produced token-major over 128-token tiles (full PE width), then
    re-partitioned into per-sequence [65, C] tiles with SBUF->SBUF DMAs.
  - Biases: layer-0 biases ride in an augmented ones-row of x/W; layer
    1-3 q/k biases fuse into the PSUM->SBUF activation (per-partition);
    layer 1-3 v biases use a rank-1 ones matmul (skipped when zero).
  - The classifier and the last layer's epilogue run in fp32.
"""

import sys

if "/opt/trn_rl_repo" not in sys.path:
    sys.path.insert(0, "/opt/trn_rl_repo")

from contextlib import ExitStack

import ml_dtypes
import numpy as np

import concourse.bass as bass
import concourse.tile as tile
from concourse import bacc, mybir
from concourse.bass_utils import run_bass_kernel_spmd

BF16 = mybir.dt.bfloat16
F32 = mybir.dt.float32
AF = mybir.ActivationFunctionType
ALU = mybir.AluOpType
AX = mybir.AxisListType

C = 512
L = 65
CIN = 2053
SCALE = 1.0 / float(np.sqrt(C))
N_CORES = 8


def build_graph(n_seq=64, G=8, KI=17, vbias=True):
    """Build the per-core Bass graph.

    n_seq: sequences per core (left+right), G: sequences per chunk,
    KI: number of 128-row k-tiles for the (padded) input channel dim,
    vbias: emit the rank-1 v-bias matmuls (skip when biases are zero).
    """
    CINP = KI * 128
    T = G * L          # tokens per chunk
    H = T // 2         # free-dim half (matmul N and PSUM bank limit)
    GH = G // 2        # sequences per half
    n_chunks = n_seq // G
    n_pairs = n_seq // 2
    CT = C // 128      # 4 channel tiles
    NTT = (T + 127) // 128          # 128-token tiles per chunk (for v)
    tts = [(i * 128, min(128, T - i * 128)) for i in range(NTT)]
    assert n_seq % G == 0 and G % 2 == 0
    assert H <= 512

    nc = bacc.Bacc(None, num_swdge_queues=4)

    xT_d = nc.dram_tensor("xT", [CINP, n_seq * L], BF16, kind="ExternalInput")
    w0T_d = nc.dram_tensor("w0T", [3, CINP, C], BF16, kind="ExternalInput")
    wqT_d = nc.dram_tensor("wqT", [3, C, C], BF16, kind="ExternalInput")
    wkT_d = nc.dram_tensor("wkT", [3, C, C], BF16, kind="ExternalInput")
    wvT_d = nc.dram_tensor("wvT", [3, C, C], BF16, kind="ExternalInput")
    bqk_d = nc.dram_tensor("bqk", [128, CT, 6], F32, kind="ExternalInput")
    bvT_d = nc.dram_tensor("bvT", [3, C], BF16, kind="ExternalInput")
    w1T_d = nc.dram_tensor("w1T", [2 * C, 256], F32, kind="ExternalInput")
    b1_d = nc.dram_tensor("b1", [128, 2], F32, kind="ExternalInput")
    w2T_d = nc.dram_tensor("w2T", [256, 2], F32, kind="ExternalInput")
    b2_d = nc.dram_tensor("b2", [2, 1], F32, kind="ExternalInput")
    out_d = nc.dram_tensor("out", [2, n_pairs], F32, kind="ExternalOutput")

    with tile.TileContext(nc) as tc, ExitStack() as ctx:
        wpool = ctx.enter_context(tc.tile_pool(name="w", bufs=1))
        xpool = ctx.enter_context(tc.tile_pool(name="xt", bufs=2))
        apool = ctx.enter_context(tc.tile_pool(name="act", bufs=2))
        xlpool = ctx.enter_context(tc.tile_pool(name="xl", bufs=2))
        ps_pool = ctx.enter_context(tc.tile_pool(name="ps", bufs=1, space="PSUM"))

        # --- chunk-0 activations first: they gate the first matmuls ---
        xt0 = xpool.tile([128, KI, T], BF16, tag="xt")
        nc.sync.dma_start(
            xt0[:], xT_d[:, 0:T].rearrange("(a p) t -> p a t", p=128))

        # --- persistent weights/constants (SWDGE queues; off the sync q) ---
        w0T_sb = wpool.tile([128, 3, KI, C], BF16)
        w0T_r = w0T_d.rearrange("j (a p) c -> p j a c", p=128)
        for j in range(3):  # layer-0 weights first, per projection
            nc.gpsimd.dma_start(w0T_sb[:, j], w0T_r[:, j])
        wqT_sb = wpool.tile([128, 3, CT, C], BF16)
        nc.gpsimd.dma_start(wqT_sb[:], wqT_d.rearrange("n (a p) c -> p n a c", p=128))
        wkT_sb = wpool.tile([128, 3, CT, C], BF16)
        nc.gpsimd.dma_start(wkT_sb[:], wkT_d.rearrange("n (a p) c -> p n a c", p=128))
        wvT_sb = wpool.tile([128, 3, CT, C], BF16)
        nc.gpsimd.dma_start(wvT_sb[:], wvT_d.rearrange("n (a p) c -> p n a c", p=128))
        bqk_sb = wpool.tile([128, CT, 6], F32)
        nc.gpsimd.dma_start(bqk_sb[:], bqk_d[:])
        bvT_sb = wpool.tile([1, 3, C], BF16)
        nc.gpsimd.dma_start(bvT_sb[:], bvT_d[None, :, :])
        w1T_sb = wpool.tile([128, 8, 256], F32)
        nc.gpsimd.dma_start(w1T_sb[:], w1T_d.rearrange("(a p) m -> p a m", p=128))
        b1_sb = wpool.tile([128, 2], F32)
        nc.gpsimd.dma_start(b1_sb[:], b1_d[:])
        w2T_sb = wpool.tile([128, 2, 2], F32)
        nc.gpsimd.dma_start(w2T_sb[:], w2T_d.rearrange("(a p) m -> p a m", p=128))
        b2_sb = wpool.tile([2, 1], F32)
        nc.gpsimd.dma_start(b2_sb[:], b2_d[:])

        onesZ = wpool.tile([65, 128], BF16)   # Z-broadcast matmul stationary
        nc.vector.memset(onesZ[:], 1.0)
        ones1 = wpool.tile([1, 128], BF16)    # rank-1 v-bias stationary
        nc.vector.memset(ones1[:], 1.0)

        P_sb = wpool.tile([128, CT, n_seq], F32)
        h2_sb = wpool.tile([128, 2, n_pairs], F32)
        y_sb = wpool.tile([2, n_pairs], F32)

        def proj_v(xt_or_x, nki, wv_ap, ly):
            """Token-major v over 128-token tiles + re-partition to [65,G,C].

            xt_or_x: channel-major input [128, nki, T]; wv_ap(ki) -> [128, C].
            """
            vstage = apool.tile([128, NTT, C], BF16, tag="vs")
            for tt, (off, tw) in enumerate(tts):
                ps = ps_pool.tile([128, C], F32, tag="pjv", bufs=2)
                for ki in range(nki):
                    nc.tensor.matmul(
                        ps[0:tw, :],
                        xt_or_x[:, ki, off:off + tw],
                        wv_ap(ki),
                        start=(ki == 0),
                        stop=(ki == nki - 1 and not (vbias and ly >= 0)),
                    )
                if vbias and ly >= 0:  # rank-1 bias: ones(tw) x bv
                    nc.tensor.matmul(ps[0:tw, :], ones1[0:1, 0:tw],
                                     bvT_sb[:, ly, :], start=False, stop=True)
                vr = apool.tile([128, C], F32, tag="vr")
                nc.scalar.activation(vr[0:tw, :], ps[0:tw, :], AF.Relu, scale=0.99)
                nc.vector.scalar_tensor_tensor(
                    vstage[0:tw, tt, :], ps[0:tw, :], 0.01, vr[0:tw, :],
                    op0=ALU.mult, op1=ALU.add)
            vT = apool.tile([65, G, C], BF16, tag="vt")
            for s in range(G):
                t0 = s * L
                a, r = divmod(t0, 128)
                n1 = min(L, 128 - r)
                nc.sync.dma_start(vT[0:n1, s, :], vstage[r:r + n1, a, :])
                if n1 < L:
                    nc.sync.dma_start(vT[n1:L, s, :], vstage[0:L - n1, a + 1, :])
            return vT

        def proj_layer0(xt):
            qk = []
            for j in range(2):  # 0=q, 1=k (channel-major [C, T])
                dst = apool.tile([128, CT, T], BF16, tag=f"qk{j}")
                for ct in range(CT):
                    for h in range(2):
                        ps = ps_pool.tile([128, H], F32, tag="pj", bufs=4)
                        for ki in range(KI):
                            nc.tensor.matmul(
                                ps[:],
                                w0T_sb[:, j, ki, ct * 128:(ct + 1) * 128],
                                xt[:, ki, h * H:(h + 1) * H],
                                start=(ki == 0),
                                stop=(ki == KI - 1),
                            )
                        nc.scalar.activation(dst[:, ct, h * H:(h + 1) * H],
                                             ps[:], AF.Copy)
                qk.append(dst)
            vT = proj_v(xt, KI, lambda ki: w0T_sb[:, 2, ki, :], -1)
            return qk[0], qk[1], vT

        def proj_layer(x, ly):
            qk = []
            for j, w in enumerate((wqT_sb, wkT_sb)):
                dst = apool.tile([128, CT, T], BF16, tag=f"qk{j}")
                for ct in range(CT):
                    for h in range(2):
                        ps = ps_pool.tile([128, H], F32, tag="pj", bufs=4)
                        for ki in range(CT):
                            nc.tensor.matmul(
                                ps[:],
                                w[:, ly, ki, ct * 128:(ct + 1) * 128],
                                x[:, ki, h * H:(h + 1) * H],
                                start=(ki == 0),
                                stop=(ki == CT - 1),
                            )
                        col = j * 3 + ly
                        nc.scalar.activation(
                            dst[:, ct, h * H:(h + 1) * H], ps[:],
                            AF.Identity, bias=bqk_sb[:, ct, col:col + 1], scale=1.0,
                        )
                qk.append(dst)
            vT = proj_v(x, CT, lambda ki: wvT_sb[:, ly, ki, :], ly)
            return qk[0], qk[1], vT

        def attention(q, k, vT, x_prev, last=False):
            E = apool.tile([65, T], BF16, tag="E")
            for s in range(G):
                ps = ps_pool.tile([65, L], F32, tag="sp", bufs=2)
                for ct in range(CT):
                    nc.tensor.matmul(
                        ps[:],
                        k[:, ct, s * L:(s + 1) * L],
                        q[:, ct, s * L:(s + 1) * L],
                        start=(ct == 0),
                        stop=(ct == CT - 1),
                    )
                nc.scalar.activation(E[:65, s * L:(s + 1) * L], ps[:],
                                     AF.Exp, scale=SCALE)
            zr = apool.tile([128, T], F32, tag="zr")
            for h in range(2):
                psz = ps_pool.tile([128, H], F32, tag="pj", bufs=4)
                nc.tensor.matmul(psz[:], onesZ[:], E[:65, h * H:(h + 1) * H],
                                 start=True, stop=True)
                nc.vector.reciprocal_approx_fast(zr[:, h * H:(h + 1) * H], psz[:])
            xn = xlpool.tile([128, CT, T], F32 if last else BF16,
                             tag="xf" if last else "x", bufs=1 if last else 2)
            for ct in range(CT):
                for h in range(2):
                    ps = ps_pool.tile([128, H], F32, tag="pj", bufs=4)
                    for s4 in range(GH):
                        s = h * GH + s4
                        nc.tensor.matmul(
                            ps[:, s4 * L:(s4 + 1) * L],
                            vT[:65, s, ct * 128:(ct + 1) * 128],
                            E[:65, s * L:(s + 1) * L],
                            start=True,
                            stop=True,
                        )
                    dst = xn[:, ct, h * H:(h + 1) * H]
                    if x_prev is None:
                        nc.vector.tensor_tensor(dst, ps[:], zr[:, h * H:(h + 1) * H],
                                                op=ALU.mult)
                    else:
                        nc.vector.tensor_tensor(ps[:], ps[:], zr[:, h * H:(h + 1) * H],
                                                op=ALU.mult)
                        nc.vector.tensor_add(dst, ps[:],
                                             x_prev[:, ct, h * H:(h + 1) * H])
            return xn

        def pool_chunk(x, c):
            for ct in range(CT):
                red = apool.tile([128, G], F32, tag="red")
                nc.vector.tensor_reduce(
                    red[:], x[:, ct, :].rearrange("p (s l) -> p s l", l=L),
                    axis=AX.X, op=ALU.add,
                )
                nc.scalar.activation(P_sb[:, ct, c * G:(c + 1) * G], red[:],
                                     AF.Copy, scale=1.0 / L)

        def classifier():
            for mt in range(2):
                ps = ps_pool.tile([128, n_pairs], F32, tag="pj", bufs=4)
                for ki in range(8):
                    rhs = (P_sb[:, ki, 0:n_pairs] if ki < CT
                           else P_sb[:, ki - CT, n_pairs:2 * n_pairs])
                    nc.tensor.matmul(ps[:], w1T_sb[:, ki, mt * 128:(mt + 1) * 128],
                                     rhs, start=(ki == 0), stop=(ki == 7))
                nc.scalar.activation(h2_sb[:, mt, :], ps[:], AF.Relu,
                                     bias=b1_sb[:, mt:mt + 1], scale=1.0)
            ps = ps_pool.tile([2, n_pairs], F32, tag="sp", bufs=2)
            for ki in range(2):
                nc.tensor.matmul(ps[:], w2T_sb[:, ki, :], h2_sb[:, ki, :],
                                 start=(ki == 0), stop=(ki == 1))
            nc.scalar.activation(y_sb[:], ps[:], AF.Identity,
                                 bias=b2_sb[:], scale=1.0)
            nc.sync.dma_start(out_d[:], y_sb[:])

        for c in range(n_chunks):
            if c == 0:
                xt = xt0
            else:
                xt = xpool.tile([128, KI, T], BF16, tag="xt")
                nc.sync.dma_start(
                    xt[:],
                    xT_d[:, c * T:(c + 1) * T].rearrange("(a p) t -> p a t", p=128))
            q, k, vT = proj_layer0(xt)
            x = attention(q, k, vT, None)
            for ly in range(3):
                q, k, vT = proj_layer(x, ly)
                x = attention(q, k, vT, x, last=(ly == 2))
            pool_chunk(x, c)
        classifier()

    nc.finalize()
    return nc


def prep_weights(Wq0, bq0, Wk0, bk0, Wv0, bv0, Wq, bq, Wk, bk, Wv, bv,
                 W1, b1, W2, b2, KI=17):
    """Host-side weight prep shared by all cores."""
    bf = ml_dtypes.bfloat16
    CINP = KI * 128
    w0T = np.zeros((3, CINP, C), np.float32)
    for j, (W_, b_) in enumerate([(Wq0, bq0), (Wk0, bk0), (Wv0, bv0)]):
        w0T[j, :CIN, :] = np.asarray(W_, np.float32).T
        w0T[j, CIN, :] = np.asarray(b_, np.float32)
    wqT = np.ascontiguousarray(np.transpose(np.asarray(Wq, np.float32), (0, 2, 1)))
    wkT = np.ascontiguousarray(np.transpose(np.asarray(Wk, np.float32), (0, 2, 1)))
    wvT = np.ascontiguousarray(np.transpose(np.asarray(Wv, np.float32), (0, 2, 1)))
    # bqk[p, ct, col]: cols 0..2 = bq layers 1..3, 3..5 = bk layers 1..3
    bqk = np.zeros((128, C // 128, 6), np.float32)
    for ly in range(3):
        bqk[:, :, ly] = np.asarray(bq, np.float32)[ly].reshape(C // 128, 128).T
        bqk[:, :, 3 + ly] = np.asarray(bk, np.float32)[ly].reshape(C // 128, 128).T
    w1T = np.ascontiguousarray(np.asarray(W1, np.float32).T)
    b1h = np.ascontiguousarray(np.asarray(b1, np.float32).reshape(2, 128).T)
    w2T = np.ascontiguousarray(np.asarray(W2, np.float32).T)
    b2h = np.asarray(b2, np.float32).reshape(2, 1)
    return {
        "w0T": w0T.astype(bf),
        "wqT": wqT.astype(bf),
        "wkT": wkT.astype(bf),
        "wvT": wvT.astype(bf),
        "bqk": bqk,
        "bvT": np.asarray(bv, np.float32).astype(bf),
        "w1T": w1T,
        "b1": b1h,
        "w2T": w2T,
        "b2": b2h,
    }


def prep_xT(dl, dr, KI=17):
    """[n_pairs, L, CIN] left+right -> padded channel-major [CINP, n_seq*L] bf16.

    Sequence order: all left sequences then all right sequences.
    Row CIN is the ones-row that carries layer-0 biases.
    """
    bf = ml_dtypes.bfloat16
    CINP = KI * 128
    n_pairs = dl.shape[0]
    ntok = n_pairs * L
    xT = np.zeros((CINP, 2 * ntok), np.float32)
    xT[:CIN, :ntok] = np.asarray(dl, np.float32).reshape(ntok, CIN).T
    xT[:CIN, ntok:] = np.asarray(dr, np.float32).reshape(ntok, CIN).T
    xT[CIN, :] = 1.0
    return xT.astype(bf)


def _ensure_ntff_hook():
    """Provide antenv.axon_hooks with a ctypes NTFF profile hook if the
    image's antenv lacks it (bass_utils imports it unguarded when
    trace=True under axon)."""
    try:
        from antenv.axon_hooks import get_axon_ntff_profile_hook  # noqa: F401
        return
    except ImportError:
        pass
    import contextlib
    import ctypes
    import types

    import antenv

    mod = types.ModuleType("antenv.axon_hooks")
    holder = {"hook": None}
    mod.set_axon_ntff_profile_hook = lambda h: holder.update(hook=h)
    mod.get_axon_ntff_profile_hook = lambda: holder["hook"]
    sys.modules["antenv.axon_hooks"] = mod
    antenv.axon_hooks = mod

    so_path = "/opt/axon/libaxon_pjrt.so"
    try:
        lib = ctypes.CDLL(so_path)
    except OSError:
        return
    if not hasattr(lib, "axon_start_nrt_profile"):
        return
    lib.axon_start_nrt_profile.argtypes = [ctypes.POINTER(ctypes.c_int64),
                                           ctypes.c_size_t]
    lib.axon_start_nrt_profile.restype = ctypes.c_int64
    lib.axon_stop_nrt_profile.argtypes = [ctypes.c_char_p]
    lib.axon_stop_nrt_profile.restype = ctypes.c_int64

    @contextlib.contextmanager
    def _hook(output_dir, device_ids):
        import jax

        jax.devices()
        if device_ids:
            ids = (ctypes.c_int64 * len(device_ids))(*device_ids)
            rc = lib.axon_start_nrt_profile(ids, len(device_ids))
        else:
            rc = lib.axon_start_nrt_profile(None, 0)
        if rc != 0:
            raise RuntimeError(f"axon_start_nrt_profile rc={rc}")
        try:
            yield
        finally:
            n = lib.axon_stop_nrt_profile(str(output_dir).encode())
            print(f"ntff profile: {n} file(s) written to {output_dir}",
                  file=sys.stderr)

    holder["hook"] = _hook


_GRAPH_CACHE = {}


def _get_graph(n_seq, G, KI, vbias):
    key = (n_seq, G, KI, vbias)
    if key not in _GRAPH_CACHE:
        _GRAPH_CACHE[key] = build_graph(n_seq=n_seq, G=G, KI=KI, vbias=vbias)
    return _GRAPH_CACHE[key]


def kernel(dataleft, dataright, Wq0, bq0, Wk0, bk0, Wv0, bv0,
           Wq, bq, Wk, bk, Wv, bv, W1, b1, W2, b2):
    import os

    B = dataleft.shape[0]
    per = B // N_CORES
    vbias = bool(np.any(np.asarray(bv)))
    nc = _get_graph(n_seq=2 * per, G=8, KI=17, vbias=vbias)
    wmap = prep_weights(Wq0, bq0, Wk0, bk0, Wv0, bv0, Wq, bq, Wk, bk, Wv, bv,
                        W1, b1, W2, b2)
    in_maps = []
    for i in range(N_CORES):
        m = dict(wmap)
        m["xT"] = prep_xT(dataleft[i * per:(i + 1) * per],
                          dataright[i * per:(i + 1) * per])
        in_maps.append(m)
    trace = bool(int(os.environ.get("KERNEL_TRACE", "0")))
    if trace:
        _ensure_ntff_hook()
    res = run_bass_kernel_spmd(nc, in_maps, core_ids=list(range(N_CORES)),
                               trace=trace)
    if trace and res.exec_time_ns is not None:
        print(f"HW exec time: {res.exec_time_ns} ns")
        kernel.last_exec_time_ns = res.exec_time_ns
        kernel.last_profile = res
    out = np.concatenate([np.ascontiguousarray(r["out"].T) for r in res.results], 0)
    return out.astype(np.float32)


# revision 20
# speedup vs baseline: 1.4150x; 1.1655x over previous
"""Trainium2 Bass kernel for nn_AppearanceTrackletEmb (dense_transformer).

Pure data-parallel: batch B=256 sharded across 8 NeuronCores (32 pairs /
64 independent sequences per core). All weights replicated. Everything
(4 attention layers + pooling + classifier) runs on-chip in one NEFF.

Layout strategy (per core):
  - x is kept channel-major [C, tokens] on chip; the host pre-transposes
    the layer-0 input to [CIN_pad, tokens] (bf16) so no on-chip transposes
    are ever needed.
  - Attention uses the S' = k^T q orientation: S'[m,l]. The softmax
    denominator Z = ones^T @ E is computed with a ones-matmul whose
    stationary operand is [65, 128], which broadcasts Z across all 128
    partitions; normalization is deferred to the out2 epilogue (a DVE
    multiply by 1/Z) so the PE never waits on the softmax chain.
  - V is produced token-major over 128-token tiles (full PE width), then
    re-partitioned into per-sequence [65, C] tiles with SBUF->SBUF DMAs.
  - Biases: layer-0 biases ride in an augmented ones-row of x/W; layer
    1-3 q/k biases fuse into the PSUM->SBUF activation (per-partition);
    layer 1-3 v biases use a rank-1 ones matmul (skipped when zero).
  - The classifier and the last layer's epilogue run in fp32.
"""

import sys

if "/opt/trn_rl_repo" not in sys.path:
    sys.path.insert(0, "/opt/trn_rl_repo")

from contextlib import ExitStack

import ml_dtypes
import numpy as np

import concourse.bass as bass
import concourse.tile as tile
from concourse import bacc, mybir
from concourse.bass_utils import run_bass_kernel_spmd

BF16 = mybir.dt.bfloat16
F32 = mybir.dt.float32
AF = mybir.ActivationFunctionType
ALU = mybir.AluOpType
AX = mybir.AxisListType

C = 512
L = 65
CIN = 2053
SCALE = 1.0 / float(np.sqrt(C))
N_CORES = 8


def build_graph(n_seq=64, G=8, KI=17, vbias=True):
    """Build the per-core Bass graph.

    n_seq: sequences per core (left+right), G: sequences per chunk,
    KI: number of 128-row k-tiles for the (padded) input channel dim,
    vbias: emit the rank-1 v-bias matmuls (skip when biases are zero).
    """
    CINP = KI * 128
    T = G * L          # tokens per chunk
    H = T // 2         # free-dim half (matmul N and PSUM bank limit)
    GH = G // 2        # sequences per half
    n_chunks = n_seq // G
    n_pairs = n_seq // 2
    CT = C // 128      # 4 channel tiles
    NTT = (T + 127) // 128          # 128-token tiles per chunk (for v)
    tts = [(i * 128, min(128, T - i * 128)) for i in range(NTT)]
    assert n_seq % G == 0 and G % 2 == 0
    assert H <= 512

    nc = bacc.Bacc(None, num_swdge_queues=4)

    xT_d = nc.dram_tensor("xT", [n_chunks, 128, KI, T], BF16, kind="ExternalInput")
    w0T_d = nc.dram_tensor("w0T", [3, 128, KI, C], BF16, kind="ExternalInput")
    wqT_d = nc.dram_tensor("wqT", [3, 128, CT, C], BF16, kind="ExternalInput")
    wkT_d = nc.dram_tensor("wkT", [3, 128, CT, C], BF16, kind="ExternalInput")
    wvT_d = nc.dram_tensor("wvT", [3, 128, CT, C], BF16, kind="ExternalInput")
    bqk_d = nc.dram_tensor("bqk", [128, CT, 6], F32, kind="ExternalInput")
    bvT_d = nc.dram_tensor("bvT", [3, C], BF16, kind="ExternalInput")
    w1T_d = nc.dram_tensor("w1T", [128, 8, 256], F32, kind="ExternalInput")
    b1_d = nc.dram_tensor("b1", [128, 2], F32, kind="ExternalInput")
    w2T_d = nc.dram_tensor("w2T", [256, 2], F32, kind="ExternalInput")
    b2_d = nc.dram_tensor("b2", [2, 1], F32, kind="ExternalInput")
    out_d = nc.dram_tensor("out", [2, n_pairs], F32, kind="ExternalOutput")

    with tile.TileContext(nc) as tc, ExitStack() as ctx:
        wpool = ctx.enter_context(tc.tile_pool(name="w", bufs=1))
        xpool = ctx.enter_context(tc.tile_pool(name="xt", bufs=2))
        apool = ctx.enter_context(tc.tile_pool(name="act", bufs=2))
        xlpool = ctx.enter_context(tc.tile_pool(name="xl", bufs=2))
        ps_pool = ctx.enter_context(tc.tile_pool(name="ps", bufs=1, space="PSUM"))

        # --- startup-critical DMAs on sync, in need-order ---
        w0T_sb = wpool.tile([128, 3, KI, C], BF16)
        xt0 = xpool.tile([128, KI, T], BF16, tag="xt")
        # v projections run first, so load v weights + chunk-0 x first
        # interleave v-weights and chunk-0 x in k-tile quarters so the
        # first v matmuls can start as soon as their k-tiles land
        _qs = [(i * KI // 4, (i + 1) * KI // 4) for i in range(4)]
        _qs = [(a, b) for a, b in _qs if b > a]
        for a, b in _qs:
            nc.sync.dma_start(w0T_sb[:, 2, a:b], w0T_d[2, :, a:b])
            nc.sync.dma_start(xt0[:, a:b], xT_d[0, :, a:b])
        nc.sync.dma_start(w0T_sb[:, 0], w0T_d[0])
        nc.sync.dma_start(w0T_sb[:, 1], w0T_d[1])

        # --- remaining weights on the scalar HWDGE queue (off sync) ---
        wqT_sb = wpool.tile([128, 3, CT, C], BF16)
        nc.scalar.dma_start(wqT_sb[:], wqT_d.rearrange("n p a c -> p n a c"))
        wkT_sb = wpool.tile([128, 3, CT, C], BF16)
        nc.scalar.dma_start(wkT_sb[:], wkT_d.rearrange("n p a c -> p n a c"))
        wvT_sb = wpool.tile([128, 3, CT, C], BF16)
        nc.scalar.dma_start(wvT_sb[:], wvT_d.rearrange("n p a c -> p n a c"))
        bqk_sb = wpool.tile([128, CT, 6], F32)
        nc.scalar.dma_start(bqk_sb[:], bqk_d[:])
        bvT_sb = wpool.tile([1, 3, C], BF16)
        nc.scalar.dma_start(bvT_sb[:], bvT_d[None, :, :])
        w1T_sb = wpool.tile([128, 8, 256], F32)
        nc.scalar.dma_start(w1T_sb[:], w1T_d[:])
        b1_sb = wpool.tile([128, 2], F32)
        nc.scalar.dma_start(b1_sb[:], b1_d[:])
        w2T_sb = wpool.tile([128, 2, 2], F32)
        nc.scalar.dma_start(w2T_sb[:], w2T_d.rearrange("(a p) m -> p a m", p=128))
        b2_sb = wpool.tile([2, 1], F32)
        nc.scalar.dma_start(b2_sb[:], b2_d[:])

        onesZ = wpool.tile([65, 128], BF16)   # Z-broadcast matmul stationary
        nc.vector.memset(onesZ[:], 1.0)
        ones1 = wpool.tile([1, 128], BF16)    # rank-1 v-bias stationary
        nc.vector.memset(ones1[:], 1.0)

        P_sb = wpool.tile([128, CT, n_seq], F32)
        h2_sb = wpool.tile([128, 2, n_pairs], F32)
        y_sb = wpool.tile([2, n_pairs], F32)

        def proj_v(xt_or_x, nki, wv_ap, ly):
            """Token-major v over 128-token tiles + re-partition to [65,G,C].

            xt_or_x: channel-major input [128, nki, T]; wv_ap(ki) -> [128, C].
            """
            vstage = apool.tile([128, NTT, C], BF16, tag="vs")
            vT = apool.tile([65, G, C], BF16, tag="vt")
            for tt, (off, tw) in enumerate(tts):
                ps = ps_pool.tile([128, C], F32, tag="pjv", bufs=2)
                for ki in range(nki):
                    nc.tensor.matmul(
                        ps[0:tw, :],
                        xt_or_x[:, ki, off:off + tw],
                        wv_ap(ki),
                        start=(ki == 0),
                        stop=(ki == nki - 1 and not (vbias and ly >= 0)),
                    )
                if vbias and ly >= 0:  # rank-1 bias: ones(tw) x bv
                    nc.tensor.matmul(ps[0:tw, :], ones1[0:1, 0:tw],
                                     bvT_sb[:, ly, :], start=False, stop=True)
                vr = apool.tile([128, C], F32, tag="vr")
                nc.scalar.activation(vr[0:tw, :], ps[0:tw, :], AF.Relu, scale=0.99)
                nc.vector.scalar_tensor_tensor(
                    vstage[0:tw, tt, :], ps[0:tw, :], 0.01, vr[0:tw, :],
                    op0=ALU.mult, op1=ALU.add)
                # re-partition each sequence as soon as its last source
                # tile is evacuated
                for s in range(G):
                    t0 = s * L
                    a, r = divmod(t0, 128)
                    n1 = min(L, 128 - r)
                    last_tile = a if n1 == L else a + 1
                    if last_tile != tt:
                        continue
                    nc.sync.dma_start(vT[0:n1, s, :], vstage[r:r + n1, a, :])
                    if n1 < L:
                        nc.sync.dma_start(vT[n1:L, s, :],
                                          vstage[0:L - n1, a + 1, :])
            return vT

        def proj_layer0(xt):
            vT = proj_v(xt, KI, lambda ki: w0T_sb[:, 2, ki, :], -1)
            qk = []
            for j in range(2):  # 0=q, 1=k (channel-major [C, T])
                dst = apool.tile([128, CT, T], BF16, tag=f"qk{j}")
                for ct in range(CT):
                    for h in range(2):
                        ps = ps_pool.tile([128, H], F32, tag="pj", bufs=4)
                        for ki in range(KI):
                            nc.tensor.matmul(
                                ps[:],
                                w0T_sb[:, j, ki, ct * 128:(ct + 1) * 128],
                                xt[:, ki, h * H:(h + 1) * H],
                                start=(ki == 0),
                                stop=(ki == KI - 1),
                            )
                        if j == 0:
                            nc.scalar.activation(dst[:, ct, h * H:(h + 1) * H],
                                                 ps[:], AF.Copy)
                        else:
                            nc.vector.tensor_copy(dst[:, ct, h * H:(h + 1) * H],
                                                  ps[:])
                qk.append(dst)
            return qk[0], qk[1], vT

        def proj_layer(x, ly):
            vT = proj_v(x, CT, lambda ki: wvT_sb[:, ly, ki, :], ly)
            qk = []
            for j, w in enumerate((wqT_sb, wkT_sb)):
                dst = apool.tile([128, CT, T], BF16, tag=f"qk{j}")
                for ct in range(CT):
                    for h in range(2):
                        ps = ps_pool.tile([128, H], F32, tag="pj", bufs=4)
                        for ki in range(CT):
                            nc.tensor.matmul(
                                ps[:],
                                w[:, ly, ki, ct * 128:(ct + 1) * 128],
                                x[:, ki, h * H:(h + 1) * H],
                                start=(ki == 0),
                                stop=(ki == CT - 1),
                            )
                        col = j * 3 + ly
                        if j == 0:
                            nc.scalar.activation(
                                dst[:, ct, h * H:(h + 1) * H], ps[:],
                                AF.Identity, bias=bqk_sb[:, ct, col:col + 1],
                                scale=1.0,
                            )
                        else:
                            nc.vector.tensor_scalar_add(
                                dst[:, ct, h * H:(h + 1) * H], ps[:],
                                bqk_sb[:, ct, col:col + 1],
                            )
                qk.append(dst)
            return qk[0], qk[1], vT

        SG = min(4, G)          # sequences per S' psum group
        n_sg = G // SG

        def softmax_parts(q, k):
            E = apool.tile([65, T], BF16, tag="E")
            for g in range(n_sg):
                ps = ps_pool.tile([65, SG * L], F32, tag="sp", bufs=2)
                for s4 in range(SG):
                    s = g * SG + s4
                    for ct in range(CT):
                        nc.tensor.matmul(
                            ps[:, s4 * L:(s4 + 1) * L],
                            k[:, ct, s * L:(s + 1) * L],
                            q[:, ct, s * L:(s + 1) * L],
                            start=(ct == 0),
                            stop=(ct == CT - 1),
                        )
                nc.scalar.activation(E[:65, g * SG * L:(g + 1) * SG * L], ps[:],
                                     AF.Exp, scale=SCALE)
            zr = apool.tile([128, T], F32, tag="zr")
            for h in range(2):
                psz = ps_pool.tile([128, H], F32, tag="pj", bufs=4)
                nc.tensor.matmul(psz[:], onesZ[:], E[:65, h * H:(h + 1) * H],
                                 start=True, stop=True)
                nc.vector.reciprocal_approx_fast(zr[:, h * H:(h + 1) * H], psz[:])
            return E, zr

        def attention(q, k, vT, x_prev):
            E, zr = softmax_parts(q, k)
            xn = xlpool.tile([128, CT, T], BF16, tag="x", bufs=2)
            for ct in range(CT):
                for h in range(2):
                    ps = ps_pool.tile([128, H], F32, tag="pj", bufs=4)
                    for s4 in range(GH):
                        s = h * GH + s4
                        nc.tensor.matmul(
                            ps[:, s4 * L:(s4 + 1) * L],
                            vT[:65, s, ct * 128:(ct + 1) * 128],
                            E[:65, s * L:(s + 1) * L],
                            start=True,
                            stop=True,
                        )
                    dst = xn[:, ct, h * H:(h + 1) * H]
                    if x_prev is None:
                        nc.vector.tensor_tensor(dst, ps[:], zr[:, h * H:(h + 1) * H],
                                                op=ALU.mult)
                    else:
                        nc.vector.tensor_tensor(ps[:], ps[:], zr[:, h * H:(h + 1) * H],
                                                op=ALU.mult)
                        nc.vector.tensor_add(dst, ps[:],
                                             x_prev[:, ct, h * H:(h + 1) * H])
            return xn

        def attention_last(q, k, vT, x_prev, c):
            # mean over l commutes with out2+residual: pool the attention
            # weights instead of materializing x3.
            E, zr = softmax_parts(q, k)
            Ew = apool.tile([65, T], F32, tag="Ew")
            nc.vector.tensor_tensor(Ew[:65, :], E[:65, :], zr[0:65, :],
                                    op=ALU.mult)
            wf = apool.tile([65, G], F32, tag="wredf")
            nc.vector.tensor_reduce(
                wf[:65, :], Ew[:65, :].rearrange("p (s l) -> p s l", l=L),
                axis=AX.X, op=ALU.add)
            w = apool.tile([65, G], BF16, tag="wred")
            nc.vector.tensor_copy(w[:65, :], wf[:65, :])
            for ct in range(CT):
                ps = ps_pool.tile([128, G], F32, tag="pj", bufs=4)
                for s in range(G):
                    nc.tensor.matmul(ps[:, s:s + 1],
                                     vT[:65, s, ct * 128:(ct + 1) * 128],
                                     w[:65, s:s + 1], start=True, stop=True)
                red = apool.tile([128, G], F32, tag="red")
                nc.vector.tensor_reduce(
                    red[:], x_prev[:, ct, :].rearrange("p (s l) -> p s l", l=L),
                    axis=AX.X, op=ALU.add)
                tmp = apool.tile([128, G], F32, tag="ptmp")
                nc.vector.tensor_add(tmp[:], ps[:], red[:])
                nc.scalar.activation(P_sb[:, ct, c * G:(c + 1) * G], tmp[:],
                                     AF.Copy, scale=1.0 / L)

        def classifier():
            for mt in range(2):
                ps = ps_pool.tile([128, n_pairs], F32, tag="pj", bufs=4)
                for ki in range(8):
                    rhs = (P_sb[:, ki, 0:n_pairs] if ki < CT
                           else P_sb[:, ki - CT, n_pairs:2 * n_pairs])
                    nc.tensor.matmul(ps[:], w1T_sb[:, ki, mt * 128:(mt + 1) * 128],
                                     rhs, start=(ki == 0), stop=(ki == 7))
                nc.scalar.activation(h2_sb[:, mt, :], ps[:], AF.Relu,
                                     bias=b1_sb[:, mt:mt + 1], scale=1.0)
            ps = ps_pool.tile([2, n_pairs], F32, tag="sp", bufs=2)
            for ki in range(2):
                nc.tensor.matmul(ps[:], w2T_sb[:, ki, :], h2_sb[:, ki, :],
                                 start=(ki == 0), stop=(ki == 1))
            nc.scalar.activation(y_sb[:], ps[:], AF.Identity,
                                 bias=b2_sb[:], scale=1.0)
            nc.sync.dma_start(out_d[:], y_sb[:])

        xt = xt0
        for c in range(n_chunks):
            q, k, vT = proj_layer0(xt)
            x = attention(q, k, vT, None)
            if c + 1 < n_chunks:  # prefetch next chunk (after l0 vT DMAs)
                xt = xpool.tile([128, KI, T], BF16, tag="xt")
                nc.sync.dma_start(xt[:], xT_d[c + 1])
            for ly in range(2):
                q, k, vT = proj_layer(x, ly)
                x = attention(q, k, vT, x)
            q, k, vT = proj_layer(x, 2)
            attention_last(q, k, vT, x, c)
        classifier()

    nc.finalize()
    return nc


def prep_weights(Wq0, bq0, Wk0, bk0, Wv0, bv0, Wq, bq, Wk, bk, Wv, bv,
                 W1, b1, W2, b2, KI=17):
    """Host-side weight prep shared by all cores."""
    bf = ml_dtypes.bfloat16
    CINP = KI * 128
    w0T = np.zeros((3, CINP, C), np.float32)
    for j, (W_, b_) in enumerate([(Wq0, bq0), (Wk0, bk0), (Wv0, bv0)]):
        w0T[j, :CIN, :] = np.asarray(W_, np.float32).T
        w0T[j, CIN, :] = np.asarray(b_, np.float32)
    wqT = np.ascontiguousarray(np.transpose(np.asarray(Wq, np.float32), (0, 2, 1)))
    wkT = np.ascontiguousarray(np.transpose(np.asarray(Wk, np.float32), (0, 2, 1)))
    wvT = np.ascontiguousarray(np.transpose(np.asarray(Wv, np.float32), (0, 2, 1)))
    # bqk[p, ct, col]: cols 0..2 = bq layers 1..3, 3..5 = bk layers 1..3
    bqk = np.zeros((128, C // 128, 6), np.float32)
    for ly in range(3):
        bqk[:, :, ly] = np.asarray(bq, np.float32)[ly].reshape(C // 128, 128).T
        bqk[:, :, 3 + ly] = np.asarray(bk, np.float32)[ly].reshape(C // 128, 128).T
    w1T = np.ascontiguousarray(np.asarray(W1, np.float32).T)
    b1h = np.ascontiguousarray(np.asarray(b1, np.float32).reshape(2, 128).T)
    w2T = np.ascontiguousarray(np.asarray(W2, np.float32).T)
    b2h = np.asarray(b2, np.float32).reshape(2, 1)
    KIv = CINP // 128
    CTv = C // 128
    w0T = np.ascontiguousarray(
        w0T.reshape(3, KIv, 128, C).transpose(0, 2, 1, 3))
    wqT = np.ascontiguousarray(
        wqT.reshape(3, CTv, 128, C).transpose(0, 2, 1, 3))
    wkT = np.ascontiguousarray(
        wkT.reshape(3, CTv, 128, C).transpose(0, 2, 1, 3))
    wvT = np.ascontiguousarray(
        wvT.reshape(3, CTv, 128, C).transpose(0, 2, 1, 3))
    w1T = np.ascontiguousarray(w1T.reshape(8, 128, 256).transpose(1, 0, 2))
    return {
        "w0T": w0T.astype(bf),
        "wqT": wqT.astype(bf),
        "wkT": wkT.astype(bf),
        "wvT": wvT.astype(bf),
        "bqk": bqk,
        "bvT": np.asarray(bv, np.float32).astype(bf),
        "w1T": w1T,
        "b1": b1h,
        "w2T": w2T,
        "b2": b2h,
    }


def prep_xT(dl, dr, KI=17, G=8):
    """[n_pairs, L, CIN] left+right -> [n_chunks, 128, KI, G*L] bf16 chunks.

    Sequence order: all left sequences then all right sequences.
    Row CIN is the ones-row that carries layer-0 biases.
    """
    bf = ml_dtypes.bfloat16
    CINP = KI * 128
    n_pairs = dl.shape[0]
    ntok = n_pairs * L
    xT = np.zeros((CINP, 2 * ntok), np.float32)
    xT[:CIN, :ntok] = np.asarray(dl, np.float32).reshape(ntok, CIN).T
    xT[:CIN, ntok:] = np.asarray(dr, np.float32).reshape(ntok, CIN).T
    xT[CIN, :] = 1.0
    T = G * L
    n_chunks = (2 * ntok) // T
    # [CINP, tok] -> [n_chunks, 128, KI, T]
    out = xT.reshape(KI, 128, n_chunks, T).transpose(2, 1, 0, 3)
    return np.ascontiguousarray(out).astype(bf)


def _ensure_ntff_hook():
    """Provide antenv.axon_hooks with a ctypes NTFF profile hook if the
    image's antenv lacks it (bass_utils imports it unguarded when
    trace=True under axon)."""
    try:
        from antenv.axon_hooks import get_axon_ntff_profile_hook  # noqa: F401
        return
    except ImportError:
        pass
    import contextlib
    import ctypes
    import types

    import antenv

    mod = types.ModuleType("antenv.axon_hooks")
    holder = {"hook": None}
    mod.set_axon_ntff_profile_hook = lambda h: holder.update(hook=h)
    mod.get_axon_ntff_profile_hook = lambda: holder["hook"]
    sys.modules["antenv.axon_hooks"] = mod
    antenv.axon_hooks = mod

    so_path = "/opt/axon/libaxon_pjrt.so"
    try:
        lib = ctypes.CDLL(so_path)
    except OSError:
        return
    if not hasattr(lib, "axon_start_nrt_profile"):
        return
    lib.axon_start_nrt_profile.argtypes = [ctypes.POINTER(ctypes.c_int64),
                                           ctypes.c_size_t]
    lib.axon_start_nrt_profile.restype = ctypes.c_int64
    lib.axon_stop_nrt_profile.argtypes = [ctypes.c_char_p]
    lib.axon_stop_nrt_profile.restype = ctypes.c_int64

    @contextlib.contextmanager
    def _hook(output_dir, device_ids):
        import jax

        jax.devices()
        if device_ids:
            ids = (ctypes.c_int64 * len(device_ids))(*device_ids)
            rc = lib.axon_start_nrt_profile(ids, len(device_ids))
        else:
            rc = lib.axon_start_nrt_profile(None, 0)
        if rc != 0:
            raise RuntimeError(f"axon_start_nrt_profile rc={rc}")
        try:
            yield
        finally:
            n = lib.axon_stop_nrt_profile(str(output_dir).encode())
            print(f"ntff profile: {n} file(s) written to {output_dir}",
                  file=sys.stderr)

    holder["hook"] = _hook


_GRAPH_CACHE = {}


def _get_graph(n_seq, G, KI, vbias):
    key = (n_seq, G, KI, vbias)
    if key not in _GRAPH_CACHE:
        _GRAPH_CACHE[key] = build_graph(n_seq=n_seq, G=G, KI=KI, vbias=vbias)
    return _GRAPH_CACHE[key]


def kernel(dataleft, dataright, Wq0, bq0, Wk0, bk0, Wv0, bv0,
           Wq, bq, Wk, bk, Wv, bv, W1, b1, W2, b2):
    import os

    B = dataleft.shape[0]
    per = B // N_CORES
    vbias = bool(np.any(np.asarray(bv)))
    nc = _get_graph(n_seq=2 * per, G=8, KI=17, vbias=vbias)
    wmap = prep_weights(Wq0, bq0, Wk0, bk0, Wv0, bv0, Wq, bq, Wk, bk, Wv, bv,
                        W1, b1, W2, b2)
    in_maps = []
    for i in range(N_CORES):
        m = dict(wmap)
        m["xT"] = prep_xT(dataleft[i * per:(i + 1) * per],
                          dataright[i * per:(i + 1) * per])
        in_maps.append(m)
    trace = bool(int(os.environ.get("KERNEL_TRACE", "0")))
    if trace:
        _ensure_ntff_hook()
    res = run_bass_kernel_spmd(nc, in_maps, core_ids=list(range(N_CORES)),
                               trace=trace)
    if trace and res.exec_time_ns is not None:
        print(f"HW exec time: {res.exec_time_ns} ns")
        kernel.last_exec_time_ns = res.exec_time_ns
        kernel.last_profile = res
    out = np.concatenate([np.ascontiguousarray(r["out"].T) for r in res.results], 0)
    return out.astype(np.float32)
